# revision 8
# baseline (speedup 1.0000x reference)
"""BiMambaBlock Trainium2 kernel — 8-core SPMD.

Sharding: core = dir*4 + b*2 + half  (dir: fwd/bwd mamba, b: batch, half: d_inner half).
Each core computes one direction's Mamba for one batch element over 768 of the 1536
d_inner channels (in-proj for the xc path is duplicated across the half pair so the
dbc projection needs no mid-kernel collective), produces its partial contribution to
the output projection (out_w and proj_w folded into one matrix), un-flips it for the
bwd direction via predicated DMA writes, AllReduces over the 4 cores of each batch
element, and applies the residual + LayerNorm redundantly.

Everything flows in channels-on-partitions [c, t] layout; the selective scan runs as
DVE tensor_tensor_scan (state = dA*state + u) with time on the free dimension.
"""

import sys
import numpy as np

for _p in ("/opt/trn_rl_repo",):
    if _p not in sys.path:
        sys.path.insert(0, _p)

B, L, D = 2, 2048, 768
E = 2
DI = E * D            # 1536
HDI = DI // 2         # 768 channels per core
S = 16
KCONV = 4
R = 48

NKT = 6               # k-tiles of D (768/128)
NCT_XC = 12           # c-tiles of full DI (xc path)
NCT = 18              # 12 xc + 6 z(half)
NCB = 6               # c-tiles of the core's half (768/128)
NTCH = 4              # 512-wide time chunks
NTT = 16              # 128-token tiles
TCH = 512

_CACHE = {}


def _build(cfg):
    import concourse.bacc as bacc
    import concourse.mybir as mybir
    import concourse.tile as tile

    DT = mybir.dt
    F32, F16 = DT.float32, DT.float16
    AL = mybir.AluOpType
    AF = mybir.ActivationFunctionType

    nc = bacc.Bacc("TRN2", target_bir_lowering=False, debug=False, num_devices=8)

    def din(name, shape, dt=F32):
        return nc.declare_dram_parameter(name, list(shape), dt, isOutput=False)

    # ---------------- inputs (per-core views, host-prepped) ----------------
    xT = din("xT", [NKT, 128, L], F16)            # x[b].T (time-flipped if bwd), k-chunked
    x_ln = din("x_ln", [L, D], F32)               # unflipped x[b] for the residual
    inw = din("inw", [NCT, NKT, 128, 128], F16)   # in-proj lhsT tiles [ct][k][krow, m]
    convw = din("convw", [128, NCT_XC * KCONV], F32)
    convb = din("convb", [128, NCT_XC], F32)
    xprj = din("xprj", [NCT_XC, 128, 80], F16)    # xproj lhsT per c k-tile
    dtw = din("dtw", [48, HDI], F16)              # dt lhsT [r, c_half]
    dtb = din("dtb", [128, NCB], F32)
    Aw = din("Aw", [128, NCB * S], F32)           # A[c,s] for the half, c-tiled
    Dpw = din("Dpw", [128, NCB], F32)
    MT = din("MT", [NCB, 128, D], F16)            # folded out-proj rhs per c-tile
    pb4 = din("pb4", [1, D], F16)                 # proj_b / 4
    ones1 = din("ones1", [1, 128], F16)
    Jrev = din("Jrev", [128, 128], F32)    # anti-identity for bwd time flip
    lnre = din("lnre", [128, 2 * D], F32)         # [ln_g_rep | ln_b_rep]

    out = nc.declare_dram_parameter("out", [L, D], F32, isOutput=True)

    probes = {}
    if cfg.get("debug"):
        probes["p_xc"] = nc.declare_dram_parameter("p_xc", [NCB, 128, L], F16, isOutput=True)
        probes["p_dbc"] = nc.declare_dram_parameter("p_dbc", [80, L], F16, isOutput=True)
        probes["p_dt"] = nc.declare_dram_parameter("p_dt", [NCB, 128, L], F16, isOutput=True)
        probes["p_y"] = nc.declare_dram_parameter("p_y", [NCB, 128, L], F16, isOutput=True)
        probes["p_pre"] = nc.declare_dram_parameter("p_pre", [L, D], F32, isOutput=True)

    # internal DRAM scratch
    zstash = nc.dram_tensor("zstash", [NCB, 128, L], F16)
    bcst = nc.dram_tensor("bcst", [2 * S, L], F16)
    pre = nc.dram_tensor("pre", [L, D], F32)
    post = nc.dram_tensor("post", [L, D], F32)

    with tile.TileContext(nc) as tc:
        pid = nc.sync.partition_id()
        is_fwd = pid < 4
        is_bwd = pid >= 4

        with tc.tile_pool(name="const", bufs=1) as constp, \
             tc.tile_pool(name="dtp", bufs=1) as dtp, \
             tc.tile_pool(name="dtxp", bufs=1) as dtxp, \
             tc.tile_pool(name="yp", bufs=1) as yp:

            # ---- resident constants
            xt_sb = constp.tile([128, NKT * L], F16, name="xt_sb")
            for k in range(NKT):
                nc.sync.dma_start(xt_sb[:, k * L:(k + 1) * L], xT[k])
            convw_sb = constp.tile([128, NCT_XC * KCONV], F32, name="convw_sb")
            nc.sync.dma_start(convw_sb[:], convw[:])
            convb_sb = constp.tile([128, NCT_XC], F32, name="convb_sb")
            nc.sync.dma_start(convb_sb[:], convb[:])
            xprj_sb = constp.tile([128, NCT_XC * 80], F16, name="xprj_sb")
            for k in range(NCT_XC):
                nc.sync.dma_start(xprj_sb[:, k * 80:(k + 1) * 80], xprj[k])
            dtw_sb = constp.tile([128, HDI], F16, name="dtw_sb", padded_shape=[128, HDI])
            nc.sync.dma_start(dtw_sb[0:48, :], dtw[:])
            dtb_sb = constp.tile([128, NCB], F32, name="dtb_sb")
            nc.sync.dma_start(dtb_sb[:], dtb[:])
            A_sb = constp.tile([128, NCB * S], F32, name="A_sb")
            nc.sync.dma_start(A_sb[:], Aw[:])
            Dp_sb = constp.tile([128, NCB], F32, name="Dp_sb")
            nc.sync.dma_start(Dp_sb[:], Dpw[:])
            mt_sb = constp.tile([128, NCB * D], F16, name="mt_sb")
            for k in range(NCB):
                nc.sync.dma_start(mt_sb[:, k * D:(k + 1) * D], MT[k])
            pb4_sb = constp.tile([1, D], F16, name="pb4_sb")
            nc.sync.dma_start(pb4_sb[:], pb4[:])
            ones_sb = constp.tile([1, 128], F16, name="ones_sb")
            nc.sync.dma_start(ones_sb[:], ones1[:])
            lnre_sb = constp.tile([128, 2 * D], F32, name="lnre_sb")
            nc.sync.dma_start(lnre_sb[:], lnre[:])
            jrev_sb = constp.tile([128, 128], F32, name="jrev_sb")
            nc.sync.dma_start(jrev_sb[:], Jrev[:])

            dt_sb = dtp.tile([128, NCB * L], F16, name="dt_sb")
            dtx_sb = dtxp.tile([128, NCB * L], F16, name="dtx_sb")
            y_sb = yp.tile([128, NCB * L], F16, name="y_sb")

            with tc.tile_pool(name="xcp", bufs=1) as xcp, \
                 tc.tile_pool(name="wstream", bufs=4) as wsp, \
                 tc.tile_pool(name="stage1", bufs=2) as st1, \
                 tc.tile_pool(name="psumA", bufs=2, space="PSUM") as psA:

                xc_sb = xcp.tile([128, NCT_XC * L], F16, name="xc_sb")

                # ---------------- P1: in-proj + conv + silu ----------------
                for ct in range(NCT):
                    w_t = wsp.tile([128, NKT * 128], F16, name="w_t", tag="w_t")
                    for k in range(NKT):
                        nc.sync.dma_start(w_t[:, k * 128:(k + 1) * 128], inw[ct, k])
                    ps = psA.tile([128, L], F32, name="ps_xz", tag="psA")
                    for k in range(NKT):
                        for tch in range(NTCH):
                            nc.tensor.matmul(
                                ps[:, tch * TCH:(tch + 1) * TCH],
                                w_t[:, k * 128:(k + 1) * 128],
                                xt_sb[:, k * L + tch * TCH: k * L + (tch + 1) * TCH],
                                start=(k == 0), stop=(k == NKT - 1),
                            )
                    if ct < NCT_XC:
                        # causal depthwise conv, taps j=0..3; tap 3 has shift 0
                        acc = st1.tile([128, L], F32, name="acc", tag="acc")
                        nc.vector.tensor_scalar(
                            acc[:], ps[:], convw_sb[:, ct * KCONV + 3: ct * KCONV + 4],
                            None, AL.mult)
                        for j in range(3):
                            sh = 3 - j
                            nc.vector.scalar_tensor_tensor(
                                acc[:, sh:], ps[:, :L - sh],
                                convw_sb[:, ct * KCONV + j: ct * KCONV + j + 1],
                                acc[:, sh:], AL.mult, AL.add)
                        nc.scalar.activation(
                            xc_sb[:, ct * L:(ct + 1) * L], acc[:], AF.Silu,
                            bias=convb_sb[:, ct:ct + 1], scale=1.0)
                    else:
                        zt = st1.tile([128, L], F16, name="zt", tag="zt")
                        nc.scalar.activation(zt[:], ps[:], AF.Silu)
                        nc.sync.dma_start(zstash[ct - NCT_XC], zt[:])

                # ---------------- P2: dbc_T = xproj_w @ xc_conv ----------------
                psd = psA.tile([80, L], F32, name="psd", tag="psA")
                for k in range(NCT_XC):
                    for tch in range(NTCH):
                        nc.tensor.matmul(
                            psd[:, tch * TCH:(tch + 1) * TCH],
                            xprj_sb[:, k * 80:(k + 1) * 80],
                            xc_sb[:, k * L + tch * TCH: k * L + (tch + 1) * TCH],
                            start=(k == 0), stop=(k == NCT_XC - 1),
                        )
                dbc_sb = st1.tile([80, L], F16, name="dbc_sb", tag="dbc", bufs=1)
                nc.scalar.activation(dbc_sb[:], psd[:], AF.Copy)
                nc.sync.dma_start(bcst[:], dbc_sb[48:80, :])
                if cfg.get("debug"):
                    nc.sync.dma_start(probes["p_dbc"][:], dbc_sb[:])

                # ---------------- P3: dt = softplus(dbc[:48] @ dtw + dtb); dtx; y-init ----------------
                for cb in range(NCB):
                    psdt = psA.tile([128, L], F32, name="psdt", tag="psA")
                    for tch in range(NTCH):
                        nc.tensor.matmul(
                            psdt[:, tch * TCH:(tch + 1) * TCH],
                            dtw_sb[0:48, cb * 128:(cb + 1) * 128],
                            dbc_sb[0:48, tch * TCH:(tch + 1) * TCH],
                            start=True, stop=True,
                        )
                    spe = st1.tile([128, L], F32, name="spe", tag="acc")
                    nc.scalar.activation(spe[:], psdt[:], AF.Exp,
                                         bias=dtb_sb[:, cb:cb + 1], scale=1.0)
                    nc.scalar.activation(
                        dt_sb[:, cb * L:(cb + 1) * L], spe[:], AF.Ln,
                        bias=1.0, scale=1.0)
                    xc_half = xc_sb[:, (cfg["half_off"] + cb) * L:(cfg["half_off"] + cb + 1) * L]
                    nc.vector.tensor_tensor(
                        dtx_sb[:, cb * L:(cb + 1) * L],
                        dt_sb[:, cb * L:(cb + 1) * L], xc_half, AL.mult)
                    nc.vector.tensor_scalar(
                        y_sb[:, cb * L:(cb + 1) * L], xc_half,
                        Dp_sb[:, cb:cb + 1], None, AL.mult)
                    if cfg.get("debug"):
                        nc.sync.dma_start(probes["p_xc"][cb], xc_half)
                        nc.sync.dma_start(probes["p_dt"][cb], dt_sb[:, cb * L:(cb + 1) * L])

            # ---------------- P4: selective scan over s ----------------
            with tc.tile_pool(name="scanw", bufs=cfg.get("scan_bufs", 3)) as swp:
                for s in range(S):
                    brep = swp.tile([128, L], F16, name="brep", tag="brep")
                    nc.sync.dma_start(brep[:], bcst[s:s + 1, :].partition_broadcast(128))
                    crep = swp.tile([128, L], F16, name="crep", tag="crep")
                    nc.sync.dma_start(crep[:], bcst[S + s:S + s + 1, :].partition_broadcast(128))
                    for cb in range(NCB):
                        dA = swp.tile([128, L], F32, name="dA", tag="dA")
                        nc.scalar.activation(
                            dA[:], dt_sb[:, cb * L:(cb + 1) * L], AF.Exp,
                            scale=A_sb[:, cb * S + s: cb * S + s + 1])
                        u = swp.tile([128, L], F16, name="u", tag="u")
                        nc.vector.tensor_tensor(
                            u[:], dtx_sb[:, cb * L:(cb + 1) * L], brep[:], AL.mult)
                        h = swp.tile([128, L], F16, name="h", tag="h")
                        nc.vector.tensor_tensor_scan(
                            h[:], dA[:], u[:], 0.0, AL.mult, AL.add)
                        p = swp.tile([128, L], F16, name="p", tag="p")
                        tt_p = nc.gpsimd if cfg.get("p_on_gp") else nc.vector
                        tt_p.tensor_tensor(p[:], h[:], crep[:], AL.mult)
                        tt_y = nc.gpsimd if cfg.get("y_on_gp") else nc.vector
                        tt_y.tensor_tensor(
                            y_sb[:, cb * L:(cb + 1) * L],
                            y_sb[:, cb * L:(cb + 1) * L], p[:], AL.add)

            if cfg.get("debug"):
                for cb in range(NCB):
                    nc.sync.dma_start(probes["p_y"][cb], y_sb[:, cb * L:(cb + 1) * L])

            # ---------------- P5: gate + folded out-proj, predicated writes ----------------
            with tc.tile_pool(name="gatep", bufs=3) as gp, \
                 tc.tile_pool(name="psumO", bufs=2, space="PSUM") as psO:
                for cb in range(NCB):
                    zt = gp.tile([128, L], F16, name="zt2", tag="zt2")
                    nc.sync.dma_start(zt[:], zstash[cb])
                    nc.vector.tensor_tensor(
                        y_sb[:, cb * L:(cb + 1) * L],
                        y_sb[:, cb * L:(cb + 1) * L], zt[:], AL.mult)
                for tt in range(NTT):
                    po = psO.tile([128, D], F32, name="po", tag="psO")
                    for nch, n0 in ((512, 0), (256, 512)):
                        nc.tensor.matmul(po[:, n0:n0 + nch], ones_sb[:],
                                         pb4_sb[:, n0:n0 + nch], start=True, stop=False)
                    for cb in range(NCB):
                        for nch, n0 in ((512, 0), (256, 512)):
                            nc.tensor.matmul(
                                po[:, n0:n0 + nch],
                                y_sb[:, cb * L + tt * 128: cb * L + (tt + 1) * 128],
                                mt_sb[:, cb * D + n0: cb * D + n0 + nch],
                                start=False, stop=(cb == NCB - 1),
                            )
                    st = gp.tile([128, D], F32, name="st_o", tag="st_o")
                    nc.scalar.activation(st[:], po[:], AF.Copy)
                    # bwd cores need partial[2047 - r]: reverse rows with the
                    # anti-identity on PE, then write to the mirrored tile.
                    po2 = psO.tile([128, D], F32, name="po2", tag="psO2")
                    for nch, n0 in ((512, 0), (256, 512)):
                        nc.tensor.matmul(po2[:, n0:n0 + nch], jrev_sb[:],
                                         st[:, n0:n0 + nch], start=True, stop=True)
                    st2 = gp.tile([128, D], F32, name="st2_o", tag="st2_o")
                    nc.scalar.activation(st2[:], po2[:], AF.Copy)
                    nc.sync.dma_start(
                        pre[tt * 128:(tt + 1) * 128, :], st[:], cond=is_fwd)
                    mtt = NTT - 1 - tt
                    nc.sync.dma_start(
                        pre[mtt * 128:(mtt + 1) * 128, :], st2[:], cond=is_bwd)

            # ---------------- P6: AllReduce over the 4 cores of each batch ----------------
            nc.gpsimd.collective_compute(
                "AllReduce", mybir.AluOpType.add,
                replica_groups=[[0, 1, 4, 5], [2, 3, 6, 7]],
                ins=[pre[:]], outs=[post[:]],
            )

            # ---------------- P7: residual + LayerNorm ----------------
            with tc.tile_pool(name="lnp", bufs=3) as lp:
                for tt in range(NTT):
                    ar = lp.tile([128, D], F32, name="ar", tag="ar")
                    nc.sync.dma_start(ar[:], post[tt * 128:(tt + 1) * 128, :])
                    xl = lp.tile([128, D], F32, name="xl", tag="xl")
                    nc.sync.dma_start(xl[:], x_ln[tt * 128:(tt + 1) * 128, :])
                    ht = lp.tile([128, D], F32, name="ht", tag="ht")
                    nc.vector.tensor_tensor(ht[:], ar[:], xl[:], AL.add)
                    if cfg.get("debug"):
                        nc.sync.dma_start(probes["p_pre"][tt * 128:(tt + 1) * 128, :], ht[:])
                    sums = lp.tile([128, 1], F32, name="sums", tag="sums")
                    sq = lp.tile([128, D], F32, name="sq", tag="sq")
                    sqs = lp.tile([128, 1], F32, name="sqs", tag="sqs")
                    nc.scalar.activation(sq[:], ht[:], AF.Square, accum_out=sqs[:])
                    nc.vector.tensor_reduce(sums[:], ht[:], mybir.AxisListType.X, AL.add)
                    mu = lp.tile([128, 1], F32, name="mu", tag="mu")
                    nc.vector.tensor_scalar(mu[:], sums[:], 1.0 / D, None, AL.mult)
                    ex2 = lp.tile([128, 1], F32, name="ex2", tag="ex2")
                    nc.vector.tensor_scalar(ex2[:], sqs[:], 1.0 / D, None, AL.mult)
                    var = lp.tile([128, 1], F32, name="var", tag="var")
                    nc.vector.scalar_tensor_tensor(
                        var[:], mu[:], -1.0, mu[:], AL.mult, AL.mult)
                    nc.vector.tensor_tensor(var[:], ex2[:], var[:], AL.add)
                    nc.vector.tensor_scalar(var[:], var[:], 1e-5, None, AL.add)
                    std = lp.tile([128, 1], F32, name="std", tag="std")
                    nc.scalar.activation(std[:], var[:], AF.Sqrt)
                    rstd = lp.tile([128, 1], F32, name="rstd", tag="rstd")
                    nc.vector.reciprocal(rstd[:], std[:])
                    nmr = lp.tile([128, 1], F32, name="nmr", tag="nmr")
                    nc.vector.scalar_tensor_tensor(
                        nmr[:], mu[:], -1.0, rstd[:], AL.mult, AL.mult)
                    nrm = lp.tile([128, D], F32, name="nrm", tag="nrm")
                    nc.scalar.activation(nrm[:], ht[:], AF.Identity,
                                         bias=nmr[:], scale=rstd[:])
                    og = lp.tile([128, D], F32, name="og", tag="og")
                    nc.vector.tensor_tensor(og[:], nrm[:], lnre_sb[:, 0:D], AL.mult)
                    nc.vector.tensor_tensor(og[:], og[:], lnre_sb[:, D:2 * D], AL.add)
                    nc.sync.dma_start(out[tt * 128:(tt + 1) * 128, :], og[:])

    nc.compile()
    return nc


def _prep_core_inputs(inputs, dirn, b, half):
    """Host-side prep of one core's input map."""
    f = np.float32
    pre = "f_" if dirn == 0 else "b_"
    in_w = inputs[pre + "in_w"].astype(f)
    conv_w = inputs[pre + "conv_w"].astype(f)
    conv_b = inputs[pre + "conv_b"].astype(f)
    xproj_w = inputs[pre + "xproj_w"].astype(f)
    dt_w = inputs[pre + "dt_w"].astype(f)
    dt_b = inputs[pre + "dt_b"].astype(f)
    A_log = inputs[pre + "A_log"].astype(f)
    Dp = inputs[pre + "Dp"].astype(f)
    out_w = inputs[pre + "out_w"].astype(f)
    proj_w = inputs["proj_w"].astype(f)
    proj_b = inputs["proj_b"].astype(f)
    ln_g = inputs["ln_g"].astype(f)
    ln_b = inputs["ln_b"].astype(f)
    x = inputs["x"][b].astype(f)                      # [L, D]

    hs, he = half * HDI, (half + 1) * HDI
    # xc c-tile permutation: core's own half first (program always uses tiles 0..5)
    perm = list(range(half * NCB, half * NCB + NCB)) + list(range((1 - half) * NCB, (1 - half) * NCB + NCB))

    xd = x[::-1] if dirn == 1 else x                  # direction-local time order
    xT = np.ascontiguousarray(xd.T)                   # [D, L]
    xT_t = xT.reshape(NKT, 128, L).astype(np.float16)

    # in-proj lhsT tiles: W = [xc rows (1536); z rows (this half)] -> W.T [768, 2304]
    xc_rows = in_w[:DI].reshape(NCT_XC, 128, D)[perm].reshape(DI, D)
    Wrows = np.concatenate([xc_rows, in_w[DI + hs: DI + he]], axis=0)  # [2304, 768]
    WT = np.ascontiguousarray(Wrows.T)                # [768, 2304]
    inw_t = np.empty((NCT, NKT, 128, 128), np.float16)
    for ct in range(NCT):
        for k in range(NKT):
            inw_t[ct, k] = WT[k * 128:(k + 1) * 128, ct * 128:(ct + 1) * 128]

    convw_t = np.empty((128, NCT_XC * KCONV), f)
    convb_t = np.empty((128, NCT_XC), f)
    for ct in range(NCT_XC):
        cs = perm[ct] * 128
        convw_t[:, ct * KCONV:(ct + 1) * KCONV] = conv_w[cs:cs + 128]
        convb_t[:, ct] = conv_b[cs:cs + 128]

    xprjT = np.ascontiguousarray(xproj_w.T)           # [1536, 80]
    xprj_t = xprjT.reshape(NCT_XC, 128, 80)[perm].astype(np.float16)

    dtw_t = np.ascontiguousarray(dt_w[hs:he].T).astype(np.float16)  # [48, 768]
    dtb_t = np.empty((128, NCB), f)
    A_t = np.empty((128, NCB * S), f)
    Dp_t = np.empty((128, NCB), f)
    A_half = -np.exp(A_log[hs:he])                    # [768, S]
    for cb in range(NCB):
        dtb_t[:, cb] = dt_b[hs + cb * 128: hs + (cb + 1) * 128]
        Dp_t[:, cb] = Dp[hs + cb * 128: hs + (cb + 1) * 128]
        A_t[:, cb * S:(cb + 1) * S] = A_half[cb * 128:(cb + 1) * 128]

    # fold out_w (this half's columns) with proj_w (this direction's columns)
    M = proj_w[:, dirn * D:(dirn + 1) * D] @ out_w[:, hs:he]   # [768 out, 768 c]
    MT_t = np.ascontiguousarray(M.T).reshape(NCB, 128, D).astype(np.float16)

    lnre_t = np.empty((128, 2 * D), f)
    lnre_t[:, :D] = np.tile(ln_g[None, :], (128, 1))
    lnre_t[:, D:] = np.tile(ln_b[None, :], (128, 1))

    return {
        "xT": xT_t,
        "x_ln": np.ascontiguousarray(x),
        "inw": inw_t,
        "convw": convw_t,
        "convb": convb_t,
        "xprj": xprj_t,
        "dtw": dtw_t,
        "dtb": dtb_t,
        "Aw": A_t,
        "Dpw": Dp_t,
        "MT": MT_t,
        "pb4": (proj_b[None, :] / 4.0).astype(np.float16),
        "ones1": np.ones((1, 128), np.float16),
        "Jrev": np.eye(128, dtype=np.float32)[::-1].copy(),
        "lnre": lnre_t,
    }


def _get_nc(cfg_key="default", debug=False):
    key = (cfg_key, debug)
    if key not in _CACHE:
        cfg = {
            "debug": debug,
            "half_off": 0,      # replaced per... half offset is data-independent: see below
            "scan_bufs": 2,
            "p_on_gp": False,
            "y_on_gp": False,
        }
        # half_off is which 6 of the 12 xc c-tiles belong to this core's half.
        # It differs per core (half 0 vs 1) but the PROGRAM must be identical
        # across cores. We therefore reorder the xc tiles HOST-side so that the
        # core's own half always occupies tiles 0..5. See _reorder note below.
        cfg["half_off"] = 0
        _CACHE[key] = _build(cfg)
    return _CACHE[key]


def kernel(**inputs):
    from concourse.bass_utils import run_bass_kernel_spmd

    nc = _get_nc()
    in_maps = []
    for core in range(8):
        dirn, b, half = core // 4, (core // 2) % 2, core % 2
        m = _prep_core_inputs(inputs, dirn, b, half)
        in_maps.append(m)
    res = run_bass_kernel_spmd(nc, in_maps, list(range(8)))
    outs = np.stack([res.results[0]["out"], res.results[2]["out"]], axis=0)
    return outs.astype(np.float32)


if __name__ == "__main__":
    pass


# revision 12
# speedup vs baseline: 1.3182x; 1.3182x over previous
"""BiMambaBlock Trainium2 kernel — 8-core SPMD.

Sharding: core = dir*4 + b*2 + half  (dir: fwd/bwd mamba, b: batch, half: d_inner half).
Each core computes one direction's Mamba for one batch element over 768 of the 1536
d_inner channels (in-proj for the xc path is duplicated across the half pair so the
dbc projection needs no mid-kernel collective), produces its partial contribution to
the output projection (out_w and proj_w folded into one matrix), un-flips it for the
bwd direction via predicated DMA writes, AllReduces over the 4 cores of each batch
element, and applies the residual + LayerNorm redundantly.

Everything flows in channels-on-partitions [c, t] layout; the selective scan runs as
DVE tensor_tensor_scan (state = dA*state + u) with time on the free dimension.
"""

import sys
import numpy as np

for _p in ("/opt/trn_rl_repo",):
    if _p not in sys.path:
        sys.path.insert(0, _p)

B, L, D = 2, 2048, 768
E = 2
DI = E * D            # 1536
HDI = DI // 2         # 768 channels per core
S = 16
KCONV = 4
R = 48

NKT = 6               # k-tiles of D (768/128)
NCT_XC = 12           # c-tiles of full DI (xc path)
NCT = 18              # 12 xc + 6 z(half)
NCB = 6               # c-tiles of the core's half (768/128)
NTCH = 4              # 512-wide time chunks
NTT = 16              # 128-token tiles
TCH = 512

_CACHE = {}


def _build(cfg):
    import concourse.bacc as bacc
    import concourse.mybir as mybir
    import concourse.tile as tile

    DT = mybir.dt
    F32, F16 = DT.float32, DT.float16
    AL = mybir.AluOpType
    AF = mybir.ActivationFunctionType

    nc = bacc.Bacc("TRN2", target_bir_lowering=False, debug=False, num_devices=8)

    def din(name, shape, dt=F32):
        return nc.declare_dram_parameter(name, list(shape), dt, isOutput=False)

    # ---------------- inputs (per-core views, host-prepped) ----------------
    xT = din("xT", [NKT, 128, L], F16)            # x[b].T (time-flipped if bwd), k-chunked
    x_ln = din("x_ln", [L, D], F32)               # unflipped x[b] for the residual
    inw = din("inw", [NCT, NKT, 128, 128], F16)   # in-proj lhsT tiles [ct][k][krow, m]
    convw = din("convw", [128, NCT_XC * KCONV], F32)
    convb = din("convb", [128, NCT_XC], F32)
    xprj = din("xprj", [NCT_XC, 128, 80], F16)    # xproj lhsT per c k-tile
    dtw = din("dtw", [48, HDI], F16)              # dt lhsT [r, c_half]
    dtb = din("dtb", [128, NCB], F32)
    Aw = din("Aw", [128, NCB * S], F32)           # A[c,s] for the half, c-tiled
    Dpw = din("Dpw", [128, NCB], F32)
    MT = din("MT", [NCB, 128, D], F16)            # folded out-proj rhs per c-tile
    pb4 = din("pb4", [1, D], F16)                 # proj_b / 4
    ones1 = din("ones1", [1, 128], F16)
    Jrev = din("Jrev", [128, 128], F32)    # anti-identity for bwd time flip
    lnre = din("lnre", [128, 2 * D], F32)         # [ln_g_rep | ln_b_rep]

    out = nc.declare_dram_parameter("out", [L, D], F32, isOutput=True)

    probes = {}
    if cfg.get("debug"):
        probes["p_xc"] = nc.declare_dram_parameter("p_xc", [NCB, 128, L], F16, isOutput=True)
        probes["p_dbc"] = nc.declare_dram_parameter("p_dbc", [80, L], F16, isOutput=True)
        probes["p_dt"] = nc.declare_dram_parameter("p_dt", [NCB, 128, L], F16, isOutput=True)
        probes["p_y"] = nc.declare_dram_parameter("p_y", [NCB, 128, L], F16, isOutput=True)
        probes["p_pre"] = nc.declare_dram_parameter("p_pre", [L, D], F32, isOutput=True)

    # internal DRAM scratch
    zstash = nc.dram_tensor("zstash", [NCB, 128, L], F16)
    bcst = nc.dram_tensor("bcst", [2 * S, L], F16)
    pre = nc.dram_tensor("pre", [L, D], F32)
    post = nc.dram_tensor("post", [L, D], F32)

    with tile.TileContext(nc) as tc:
        pid = nc.sync.partition_id()
        is_fwd = pid < 4
        is_bwd = pid >= 4

        with tc.tile_pool(name="const", bufs=1) as constp, \
             tc.tile_pool(name="dtp", bufs=1) as dtp, \
             tc.tile_pool(name="dtxp", bufs=1) as dtxp, \
             tc.tile_pool(name="yp", bufs=1) as yp:

            # ---- resident constants
            xt_sb = constp.tile([128, NKT * L], F16, name="xt_sb")
            for k in range(NKT):
                nc.sync.dma_start(xt_sb[:, k * L:(k + 1) * L], xT[k])
            convw_sb = constp.tile([128, NCT_XC * KCONV], F32, name="convw_sb")
            nc.sync.dma_start(convw_sb[:], convw[:])
            convb_sb = constp.tile([128, NCT_XC], F32, name="convb_sb")
            nc.sync.dma_start(convb_sb[:], convb[:])
            xprj_sb = constp.tile([128, NCT_XC * 80], F16, name="xprj_sb")
            for k in range(NCT_XC):
                nc.sync.dma_start(xprj_sb[:, k * 80:(k + 1) * 80], xprj[k])
            dtw_sb = constp.tile([128, HDI], F16, name="dtw_sb", padded_shape=[128, HDI])
            nc.sync.dma_start(dtw_sb[0:48, :], dtw[:])
            dtb_sb = constp.tile([128, NCB], F32, name="dtb_sb")
            nc.sync.dma_start(dtb_sb[:], dtb[:])
            A_sb = constp.tile([128, NCB * S], F32, name="A_sb")
            nc.sync.dma_start(A_sb[:], Aw[:])
            Dp_sb = constp.tile([128, NCB], F32, name="Dp_sb")
            nc.sync.dma_start(Dp_sb[:], Dpw[:])
            mt_sb = constp.tile([128, NCB * D], F16, name="mt_sb")
            for k in range(NCB):
                nc.sync.dma_start(mt_sb[:, k * D:(k + 1) * D], MT[k])
            pb4_sb = constp.tile([1, D], F16, name="pb4_sb")
            nc.sync.dma_start(pb4_sb[:], pb4[:])
            ones_sb = constp.tile([1, 128], F16, name="ones_sb")
            nc.sync.dma_start(ones_sb[:], ones1[:])
            lnre_sb = constp.tile([128, 2 * D], F32, name="lnre_sb")
            nc.sync.dma_start(lnre_sb[:], lnre[:])
            jrev_sb = constp.tile([128, 128], F32, name="jrev_sb")
            nc.sync.dma_start(jrev_sb[:], Jrev[:])

            dt_sb = dtp.tile([128, NCB * L], F16, name="dt_sb")
            dtx_sb = dtxp.tile([128, NCB * L], F16, name="dtx_sb")
            y_sb = yp.tile([128, NCB * L], F16, name="y_sb")

            with tc.tile_pool(name="xcp", bufs=1) as xcp, \
                 tc.tile_pool(name="wstream", bufs=4) as wsp, \
                 tc.tile_pool(name="stage1", bufs=2) as st1, \
                 tc.tile_pool(name="psumA", bufs=2, space="PSUM") as psA:

                xc_sb = xcp.tile([128, NCT_XC * L], F16, name="xc_sb")

                # ---------------- P1: in-proj + conv + silu ----------------
                for ct in range(NCT):
                    w_t = wsp.tile([128, NKT * 128], F16, name="w_t", tag="w_t")
                    for k in range(NKT):
                        nc.sync.dma_start(w_t[:, k * 128:(k + 1) * 128], inw[ct, k])
                    ps = psA.tile([128, L], F32, name="ps_xz", tag="psA")
                    for k in range(NKT):
                        for tch in range(NTCH):
                            nc.tensor.matmul(
                                ps[:, tch * TCH:(tch + 1) * TCH],
                                w_t[:, k * 128:(k + 1) * 128],
                                xt_sb[:, k * L + tch * TCH: k * L + (tch + 1) * TCH],
                                start=(k == 0), stop=(k == NKT - 1),
                            )
                    if ct < NCT_XC:
                        # causal depthwise conv on an f16 SBUF copy of the psum
                        ps16 = st1.tile([128, L], F16, name="ps16", tag="ps16")
                        nc.scalar.activation(ps16[:], ps[:], AF.Copy)
                        acc = st1.tile([128, L], F16, name="acc", tag="acc")
                        nc.vector.tensor_scalar(
                            acc[:], ps16[:], convw_sb[:, ct * KCONV + 3: ct * KCONV + 4],
                            None, AL.mult)
                        for j in range(3):
                            sh = 3 - j
                            nc.vector.scalar_tensor_tensor(
                                acc[:, sh:], ps16[:, :L - sh],
                                convw_sb[:, ct * KCONV + j: ct * KCONV + j + 1],
                                acc[:, sh:], AL.mult, AL.add)
                        nc.scalar.activation(
                            xc_sb[:, ct * L:(ct + 1) * L], acc[:], AF.Silu,
                            bias=convb_sb[:, ct:ct + 1], scale=1.0)
                    else:
                        zt = st1.tile([128, L], F16, name="zt", tag="zt")
                        nc.scalar.activation(zt[:], ps[:], AF.Silu)
                        nc.gpsimd.dma_start(zstash[ct - NCT_XC], zt[:])

                # ---------------- P2: dbc_T = xproj_w @ xc_conv ----------------
                psd = psA.tile([80, L], F32, name="psd", tag="psA")
                for k in range(NCT_XC):
                    for tch in range(NTCH):
                        nc.tensor.matmul(
                            psd[:, tch * TCH:(tch + 1) * TCH],
                            xprj_sb[:, k * 80:(k + 1) * 80],
                            xc_sb[:, k * L + tch * TCH: k * L + (tch + 1) * TCH],
                            start=(k == 0), stop=(k == NCT_XC - 1),
                        )
                dbc_sb = st1.tile([80, L], F16, name="dbc_sb", tag="dbc", bufs=1)
                nc.scalar.activation(dbc_sb[:], psd[:], AF.Copy)
                nc.sync.dma_start(bcst[:], dbc_sb[48:80, :])
                if cfg.get("debug"):
                    nc.sync.dma_start(probes["p_dbc"][:], dbc_sb[:])

                # ---------------- P3: dt = softplus(dbc[:48] @ dtw + dtb); dtx; y-init ----------------
                for cb in range(NCB):
                    psdt = psA.tile([128, L], F32, name="psdt", tag="psA")
                    for tch in range(NTCH):
                        nc.tensor.matmul(
                            psdt[:, tch * TCH:(tch + 1) * TCH],
                            dtw_sb[0:48, cb * 128:(cb + 1) * 128],
                            dbc_sb[0:48, tch * TCH:(tch + 1) * TCH],
                            start=True, stop=True,
                        )
                    spe = st1.tile([128, L], F32, name="spe", tag="acc")
                    nc.scalar.activation(spe[:], psdt[:], AF.Exp,
                                         bias=dtb_sb[:, cb:cb + 1], scale=1.0)
                    nc.scalar.activation(
                        dt_sb[:, cb * L:(cb + 1) * L], spe[:], AF.Ln,
                        bias=1.0, scale=1.0)
                    xc_half = xc_sb[:, (cfg["half_off"] + cb) * L:(cfg["half_off"] + cb + 1) * L]
                    nc.vector.tensor_tensor(
                        dtx_sb[:, cb * L:(cb + 1) * L],
                        dt_sb[:, cb * L:(cb + 1) * L], xc_half, AL.mult)
                    nc.vector.tensor_scalar(
                        y_sb[:, cb * L:(cb + 1) * L], xc_half,
                        Dp_sb[:, cb:cb + 1], None, AL.mult)
                    if cfg.get("debug"):
                        nc.sync.dma_start(probes["p_xc"][cb], xc_half)
                        nc.sync.dma_start(probes["p_dt"][cb], dt_sb[:, cb * L:(cb + 1) * L])

            # ---------------- P4: selective scan over s ----------------
            with tc.tile_pool(name="scanw", bufs=cfg.get("scan_bufs", 3)) as swp:
                for s in range(S):
                    beng = nc.scalar if (s % 2 == 0) else nc.gpsimd
                    ceng = nc.gpsimd if (s % 2 == 0) else nc.scalar
                    brep = swp.tile([128, L], F16, name="brep", tag="brep")
                    beng.dma_start(brep[:], bcst[s:s + 1, :].partition_broadcast(128))
                    crep = swp.tile([128, L], F16, name="crep", tag="crep")
                    ceng.dma_start(crep[:], bcst[S + s:S + s + 1, :].partition_broadcast(128))
                    for cb in range(NCB):
                        dA = swp.tile([128, L], F32, name="dA", tag="dA")
                        nc.scalar.activation(
                            dA[:], dt_sb[:, cb * L:(cb + 1) * L], AF.Exp,
                            scale=A_sb[:, cb * S + s: cb * S + s + 1])
                        u = swp.tile([128, L], F16, name="u", tag="u")
                        nc.vector.tensor_tensor(
                            u[:], dtx_sb[:, cb * L:(cb + 1) * L], brep[:], AL.mult)
                        h = swp.tile([128, L], F16, name="h", tag="h")
                        nc.vector.tensor_tensor_scan(
                            h[:], dA[:], u[:], 0.0, AL.mult, AL.add)
                        p = swp.tile([128, L], F16, name="p", tag="p")
                        tt_p = nc.gpsimd if cfg.get("p_on_gp") else nc.vector
                        tt_p.tensor_tensor(p[:], h[:], crep[:], AL.mult)
                        tt_y = nc.gpsimd if cfg.get("y_on_gp") else nc.vector
                        tt_y.tensor_tensor(
                            y_sb[:, cb * L:(cb + 1) * L],
                            y_sb[:, cb * L:(cb + 1) * L], p[:], AL.add)

            if cfg.get("debug"):
                for cb in range(NCB):
                    nc.sync.dma_start(probes["p_y"][cb], y_sb[:, cb * L:(cb + 1) * L])

            # ---------------- P5: gate + folded out-proj, predicated writes ----------------
            with tc.tile_pool(name="gatep", bufs=3) as gp, \
                 tc.tile_pool(name="psumO", bufs=2, space="PSUM") as psO:
                for cb in range(NCB):
                    zt = gp.tile([128, L], F16, name="zt2", tag="zt2")
                    nc.gpsimd.dma_start(zt[:], zstash[cb])
                    nc.vector.tensor_tensor(
                        y_sb[:, cb * L:(cb + 1) * L],
                        y_sb[:, cb * L:(cb + 1) * L], zt[:], AL.mult)
                def outproj_tile(tt):
                    po = psO.tile([128, D], F32, name="po", tag="psO")
                    for nch, n0 in ((512, 0), (256, 512)):
                        nc.tensor.matmul(po[:, n0:n0 + nch], ones_sb[:],
                                         pb4_sb[:, n0:n0 + nch], start=True, stop=False)
                    for cb in range(NCB):
                        for nch, n0 in ((512, 0), (256, 512)):
                            nc.tensor.matmul(
                                po[:, n0:n0 + nch],
                                y_sb[:, cb * L + tt * 128: cb * L + (tt + 1) * 128],
                                mt_sb[:, cb * D + n0: cb * D + n0 + nch],
                                start=False, stop=(cb == NCB - 1),
                            )
                    st = gp.tile([128, D], F32, name="st_o", tag="st_o")
                    nc.scalar.activation(st[:], po[:], AF.Copy)
                    # bwd cores need partial[2047 - r]: reverse rows with the
                    # anti-identity on PE, then write to the mirrored tile.
                    po2 = psO.tile([128, D], F32, name="po2", tag="psO2")
                    for nch, n0 in ((512, 0), (256, 512)):
                        nc.tensor.matmul(po2[:, n0:n0 + nch], jrev_sb[:],
                                         st[:, n0:n0 + nch], start=True, stop=True)
                    st2 = gp.tile([128, D], F32, name="st2_o", tag="st2_o")
                    nc.scalar.activation(st2[:], po2[:], AF.Copy)
                    nc.sync.dma_start(
                        pre[tt * 128:(tt + 1) * 128, :], st[:], cond=is_fwd)
                    mtt = NTT - 1 - tt
                    nc.sync.dma_start(
                        pre[mtt * 128:(mtt + 1) * 128, :], st2[:], cond=is_bwd)

                # paired order: after pairs (0,15),(1,14),(2,13),(3,12) the row
                # segments [0:512) and [1536:2048) are complete on every core.
                for i in range(8):
                    outproj_tile(i)
                    outproj_tile(NTT - 1 - i)
                nc.gpsimd.collective_compute(
                    "AllReduce", mybir.AluOpType.add,
                    replica_groups=[[0, 1, 4, 5], [2, 3, 6, 7]],
                    ins=[pre[:]], outs=[post[:]],
                )

            # ---------------- P7: residual + LayerNorm (segment order) ----------------
            with tc.tile_pool(name="lnp", bufs=3) as lp:
                ln_order = [0, 1, 2, 3, 12, 13, 14, 15, 4, 5, 6, 7, 8, 9, 10, 11]
                for tt in ln_order:
                    ar = lp.tile([128, D], F32, name="ar", tag="ar")
                    nc.scalar.dma_start(ar[:], post[tt * 128:(tt + 1) * 128, :])
                    xl = lp.tile([128, D], F32, name="xl", tag="xl")
                    nc.gpsimd.dma_start(xl[:], x_ln[tt * 128:(tt + 1) * 128, :])
                    ht = lp.tile([128, D], F32, name="ht", tag="ht")
                    nc.gpsimd.tensor_tensor(ht[:], ar[:], xl[:], AL.add)
                    if cfg.get("debug"):
                        nc.sync.dma_start(probes["p_pre"][tt * 128:(tt + 1) * 128, :], ht[:])
                    sums = lp.tile([128, 1], F32, name="sums", tag="sums")
                    sq = lp.tile([128, D], F32, name="sq", tag="sq")
                    sqs = lp.tile([128, 1], F32, name="sqs", tag="sqs")
                    nc.scalar.activation(sq[:], ht[:], AF.Square, accum_out=sqs[:])
                    nc.vector.tensor_reduce(sums[:], ht[:], mybir.AxisListType.X, AL.add)
                    mu = lp.tile([128, 1], F32, name="mu", tag="mu")
                    nc.vector.tensor_scalar(mu[:], sums[:], 1.0 / D, None, AL.mult)
                    ex2 = lp.tile([128, 1], F32, name="ex2", tag="ex2")
                    nc.vector.tensor_scalar(ex2[:], sqs[:], 1.0 / D, None, AL.mult)
                    var = lp.tile([128, 1], F32, name="var", tag="var")
                    nc.vector.scalar_tensor_tensor(
                        var[:], mu[:], -1.0, mu[:], AL.mult, AL.mult)
                    nc.vector.tensor_tensor(var[:], ex2[:], var[:], AL.add)
                    nc.vector.tensor_scalar(var[:], var[:], 1e-5, None, AL.add)
                    std = lp.tile([128, 1], F32, name="std", tag="std")
                    nc.scalar.activation(std[:], var[:], AF.Sqrt)
                    rstd = lp.tile([128, 1], F32, name="rstd", tag="rstd")
                    nc.vector.reciprocal(rstd[:], std[:])
                    nmr = lp.tile([128, 1], F32, name="nmr", tag="nmr")
                    nc.vector.scalar_tensor_tensor(
                        nmr[:], mu[:], -1.0, rstd[:], AL.mult, AL.mult)
                    nrm = lp.tile([128, D], F32, name="nrm", tag="nrm")
                    nc.scalar.activation(nrm[:], ht[:], AF.Identity,
                                         bias=nmr[:], scale=rstd[:])
                    og = lp.tile([128, D], F32, name="og", tag="og")
                    nc.gpsimd.tensor_tensor(og[:], nrm[:], lnre_sb[:, 0:D], AL.mult)
                    nc.gpsimd.tensor_tensor(og[:], og[:], lnre_sb[:, D:2 * D], AL.add)
                    nc.sync.dma_start(out[tt * 128:(tt + 1) * 128, :], og[:])

    nc.compile()
    return nc


def _prep_core_inputs(inputs, dirn, b, half):
    """Host-side prep of one core's input map."""
    f = np.float32
    pre = "f_" if dirn == 0 else "b_"
    in_w = inputs[pre + "in_w"].astype(f)
    conv_w = inputs[pre + "conv_w"].astype(f)
    conv_b = inputs[pre + "conv_b"].astype(f)
    xproj_w = inputs[pre + "xproj_w"].astype(f)
    dt_w = inputs[pre + "dt_w"].astype(f)
    dt_b = inputs[pre + "dt_b"].astype(f)
    A_log = inputs[pre + "A_log"].astype(f)
    Dp = inputs[pre + "Dp"].astype(f)
    out_w = inputs[pre + "out_w"].astype(f)
    proj_w = inputs["proj_w"].astype(f)
    proj_b = inputs["proj_b"].astype(f)
    ln_g = inputs["ln_g"].astype(f)
    ln_b = inputs["ln_b"].astype(f)
    x = inputs["x"][b].astype(f)                      # [L, D]

    hs, he = half * HDI, (half + 1) * HDI
    # xc c-tile permutation: core's own half first (program always uses tiles 0..5)
    perm = list(range(half * NCB, half * NCB + NCB)) + list(range((1 - half) * NCB, (1 - half) * NCB + NCB))

    xd = x[::-1] if dirn == 1 else x                  # direction-local time order
    xT = np.ascontiguousarray(xd.T)                   # [D, L]
    xT_t = xT.reshape(NKT, 128, L).astype(np.float16)

    # in-proj lhsT tiles: W = [xc rows (1536); z rows (this half)] -> W.T [768, 2304]
    xc_rows = in_w[:DI].reshape(NCT_XC, 128, D)[perm].reshape(DI, D)
    Wrows = np.concatenate([xc_rows, in_w[DI + hs: DI + he]], axis=0)  # [2304, 768]
    WT = np.ascontiguousarray(Wrows.T)                # [768, 2304]
    inw_t = np.empty((NCT, NKT, 128, 128), np.float16)
    for ct in range(NCT):
        for k in range(NKT):
            inw_t[ct, k] = WT[k * 128:(k + 1) * 128, ct * 128:(ct + 1) * 128]

    convw_t = np.empty((128, NCT_XC * KCONV), f)
    convb_t = np.empty((128, NCT_XC), f)
    for ct in range(NCT_XC):
        cs = perm[ct] * 128
        convw_t[:, ct * KCONV:(ct + 1) * KCONV] = conv_w[cs:cs + 128]
        convb_t[:, ct] = conv_b[cs:cs + 128]

    xprjT = np.ascontiguousarray(xproj_w.T)           # [1536, 80]
    xprj_t = xprjT.reshape(NCT_XC, 128, 80)[perm].astype(np.float16)

    dtw_t = np.ascontiguousarray(dt_w[hs:he].T).astype(np.float16)  # [48, 768]
    dtb_t = np.empty((128, NCB), f)
    A_t = np.empty((128, NCB * S), f)
    Dp_t = np.empty((128, NCB), f)
    A_half = -np.exp(A_log[hs:he])                    # [768, S]
    for cb in range(NCB):
        dtb_t[:, cb] = dt_b[hs + cb * 128: hs + (cb + 1) * 128]
        Dp_t[:, cb] = Dp[hs + cb * 128: hs + (cb + 1) * 128]
        A_t[:, cb * S:(cb + 1) * S] = A_half[cb * 128:(cb + 1) * 128]

    # fold out_w (this half's columns) with proj_w (this direction's columns)
    M = proj_w[:, dirn * D:(dirn + 1) * D] @ out_w[:, hs:he]   # [768 out, 768 c]
    MT_t = np.ascontiguousarray(M.T).reshape(NCB, 128, D).astype(np.float16)

    lnre_t = np.empty((128, 2 * D), f)
    lnre_t[:, :D] = np.tile(ln_g[None, :], (128, 1))
    lnre_t[:, D:] = np.tile(ln_b[None, :], (128, 1))

    return {
        "xT": xT_t,
        "x_ln": np.ascontiguousarray(x),
        "inw": inw_t,
        "convw": convw_t,
        "convb": convb_t,
        "xprj": xprj_t,
        "dtw": dtw_t,
        "dtb": dtb_t,
        "Aw": A_t,
        "Dpw": Dp_t,
        "MT": MT_t,
        "pb4": (proj_b[None, :] / 4.0).astype(np.float16),
        "ones1": np.ones((1, 128), np.float16),
        "Jrev": np.eye(128, dtype=np.float32)[::-1].copy(),
        "lnre": lnre_t,
    }


def _get_nc(cfg_key="default", debug=False):
    key = (cfg_key, debug)
    if key not in _CACHE:
        cfg = {
            "debug": debug,
            "half_off": 0,      # replaced per... half offset is data-independent: see below
            "scan_bufs": 2,
            "p_on_gp": False,
            "y_on_gp": False,
        }
        # half_off is which 6 of the 12 xc c-tiles belong to this core's half.
        # It differs per core (half 0 vs 1) but the PROGRAM must be identical
        # across cores. We therefore reorder the xc tiles HOST-side so that the
        # core's own half always occupies tiles 0..5. See _reorder note below.
        cfg["half_off"] = 0
        _CACHE[key] = _build(cfg)
    return _CACHE[key]


def kernel(**inputs):
    from concourse.bass_utils import run_bass_kernel_spmd

    nc = _get_nc()
    in_maps = []
    for core in range(8):
        dirn, b, half = core // 4, (core // 2) % 2, core % 2
        m = _prep_core_inputs(inputs, dirn, b, half)
        in_maps.append(m)
    res = run_bass_kernel_spmd(nc, in_maps, list(range(8)))
    outs = np.stack([res.results[0]["out"], res.results[2]["out"]], axis=0)
    return outs.astype(np.float32)


if __name__ == "__main__":
    pass
